# revision 17
# baseline (speedup 1.0000x reference)
"""ChebConv (K=4) Trainium2 Bass kernel — node-major mapping.

Problem (hardcoded): B=16 graphs, N=2048 nodes, F=64 feats, K=4, out_dim=128.
  L = D A0 D  (A0 = A with zeroed diag, D = diag(1/(eps+sqrt(rowsum(A0)))))
  T0 = X; T1 = L X; T_t = 2 L T_{t-1} - T_{t-2}
  out = relu(concat(T0..T3) @ kernel + bias)

Sharding: batch across 8 cores, 2 graphs per core; host concatenates.

Device algorithm (per core, graphs g=0,1), with V_t := 2d * T_t:
    V0      = 2d*X
    W_t     = A0 @ V_t          (pure bf16 matmul, A^T blocks as weights)
    V1      = d^2 * W0
    V_{t+1} = 2d^2 * W_t - V_{t-1}
    out     = relu( sum_t (V_t/(2d)) @ K_t + bias )

Key layout choice: the Chebyshev matmuls run NODE-major — each output tile
is [128 nodes x 64 feats] with an A^T 128x128 block as the stationary
(lhsT) operand and V as the moving rhs. Outputs are full 128 partitions
wide, so the PE streams half the rows of the feature-major alternative,
and W lands node-major so the V update is a single elementwise op (no
re-transpose). Row sums ride the PE too (ones-column matmuls against the
A^T blocks), freeing the DVE. A arrives f32 in HBM; the SWDGE DMA casts
to bf16 on the fly (2 node-chunks per DMA). A^T is built by identity
matmuls, drained PSUM->SBUF round-robin over ACT/DVE/Pool. The final
projection reads e-scaled Z^T tiles built by diag(e) matmuls; bias is
added with a rank-1 ones x bias matmul and relu rides the PSUM drain.
"""

import numpy as np

P = 128          # partitions
N = 2048         # nodes per graph
F = 64           # input features
OUT = 128        # output features
GP = 2           # graphs per core
NT = N // P      # 16 node chunks
CH = 2           # node chunks per A-load DMA
NCORES = 8

_cached = {}


def _build_nc():
    import ml_dtypes
    import concourse.bacc as bacc
    import concourse.mybir as mybir
    from concourse.tile import TileContext

    f32 = mybir.dt.float32
    bf16 = mybir.dt.bfloat16
    Alu = mybir.AluOpType
    Act = mybir.ActivationFunctionType

    nc = bacc.Bacc("TRN2", target_bir_lowering=False)

    a_in = nc.dram_tensor("a", [GP, N, N], f32, kind="ExternalInput")
    x_in = nc.dram_tensor("x", [GP, N, F], f32, kind="ExternalInput")
    wk_in = nc.dram_tensor("wk", [2 * P, OUT], f32, kind="ExternalInput")
    bias_in = nc.dram_tensor("bias", [OUT], f32, kind="ExternalInput")
    o_out = nc.dram_tensor("out", [GP, N, OUT], f32, kind="ExternalOutput")

    ident_np = np.eye(P, dtype=ml_dtypes.bfloat16)
    ident_dram = nc.inline_tensor(ident_np, name="identbf")

    with TileContext(nc) as tc, \
         tc.tile_pool(name="const", bufs=1) as const, \
         tc.tile_pool(name="big", bufs=1) as big, \
         tc.tile_pool(name="astage", bufs=3) as astage, \
         tc.tile_pool(name="dch", bufs=2) as dch, \
         tc.tile_pool(name="dgep", bufs=4) as dgep, \
         tc.tile_pool(name="outs", bufs=2) as outs, \
         tc.tile_pool(name="ps_tr", bufs=2, space="PSUM") as ps_tr, \
         tc.tile_pool(name="ps_it", bufs=3, space="PSUM") as ps_it, \
         tc.tile_pool(name="ps_rs", bufs=1, space="PSUM") as ps_rs, \
         tc.tile_pool(name="ps_z", bufs=2, space="PSUM") as ps_z:

        # ---- constants -------------------------------------------------
        ident = const.tile([P, P], bf16)
        nc.sync.dma_start(out=ident, in_=ident_dram[:, :])
        mask = const.tile([P, P], bf16)   # 1 - I
        nc.vector.tensor_scalar(mask, ident, -1.0, 1.0, Alu.mult, Alu.add)
        kab = const.tile([P, OUT], bf16)
        kcd = const.tile([P, OUT], bf16)
        kstage = const.tile([P, 2 * OUT], f32)
        nc.sync.dma_start(out=kstage[:, 0:OUT], in_=wk_in[0:P, :])
        nc.sync.dma_start(out=kstage[:, OUT : 2 * OUT], in_=wk_in[P : 2 * P, :])
        nc.vector.tensor_copy(kab, kstage[:, 0:OUT])
        nc.vector.tensor_copy(kcd, kstage[:, OUT : 2 * OUT])
        bias_f32 = const.tile([1, OUT], f32)
        nc.sync.dma_start(out=bias_f32, in_=bias_in[None, :])
        bias4 = const.tile([1, 4 * OUT], bf16)   # bias tiled 4x along free
        for j in range(4):
            nc.vector.tensor_copy(bias4[:, j * OUT : (j + 1) * OUT], bias_f32)
        ones_row = const.tile([1, P], bf16)
        nc.vector.memset(ones_row, 1.0)
        ones_col = const.tile([P, 1], bf16)
        nc.vector.memset(ones_col, 1.0)

        # ---- persistent SBUF state ------------------------------------
        # A^T per graph: [:, q, :] is j-tile q (j = 128q+p), free = node i
        at = [big.tile([P, NT, N], bf16, name=f"at{g}") for g in range(GP)]
        # V pairs, node-major: [:, c, 0:64] = V_t even, [:, c, 64:128] = odd
        zp01 = [big.tile([P, NT, 2 * F], bf16, name=f"zp01_{g}") for g in range(GP)]
        zp23 = [big.tile([P, NT, 2 * F], bf16, name=f"zp23_{g}") for g in range(GP)]
        # e-scaled Z^T pairs for the projection (feature-major)
        ztab = [big.tile([P, N], bf16, name=f"ztab{g}") for g in range(GP)]
        ztcd = [big.tile([P, N], bf16, name=f"ztcd{g}") for g in range(GP)]
        xst = [big.tile([P, NT, F], f32, name=f"xst{g}") for g in range(GP)]
        dsq = big.tile([P, NT, GP], f32, name="dsq")   # d^2
        d2s = big.tile([P, NT, GP], f32, name="d2s")   # 2 d^2
        eh = big.tile([P, NT, GP], f32, name="eh")     # 1/(2d)
        dbl = big.tile([P, NT, GP], f32, name="dbl")   # 2d

        # all 32 per-(graph,chunk) rowsum accumulators live in one bank
        rs_ps = ps_rs.tile([P, 512], f32, name="rsps")

        drain_rr = [0]

        def drain(dst, src):
            k = drain_rr[0] % 2
            drain_rr[0] += 1
            if k == 1:
                nc.vector.tensor_copy(out=dst, in_=src)
            else:
                nc.scalar.copy(out=dst, in_=src)

        def reg(w, i):
            return w[i // 8][:, (i % 8) * F : (i % 8 + 1) * F]

        # PSUM pending-zero is tracked per bank (2KB per partition): only the
        # FIRST write to a bank may carry start=True; later writes to
        # still-pending bytes replace, to cleared bytes accumulate.
        rs_first = [True]

        def emit_rowsum(g, c):
            col = g * NT + c
            for q in range(NT):
                nc.tensor.matmul(
                    rs_ps[:, col : col + 1],
                    lhsT=at[g][:, q, c * P : (c + 1) * P],
                    rhs=ones_col,
                    start=rs_first[0], stop=(q == NT - 1),
                    skip_group_check=True,
                )
                rs_first[0] = False

        it_ps = {}

        # ---- load phase: stream A, cast, transpose, rowsum, d, V0, it1 -
        # Emission is a lagged software pipeline: every consumer is emitted
        # far enough behind its producer that its deps are met at dispatch,
        # so the 4-deep per-engine wait queues never head-of-line block.
        XBAR = {0, 4, 8, 12}   # chunks transposed by the DMA XBAR, not PE

        def load_phase(g):
            nc.sync.dma_start(
                out=xst[g], in_=x_in[g].rearrange("(c p) f -> p c f", p=P)
            )
            w1 = [ps_it.tile([P, 512], f32, name=f"w1_{g}{h}", tag="it")
                  for h in range(2)]
            it_ps[g] = w1
            cnt = [0] * NT
            bank_first = [True, True]
            pend_rs = []
            uchs = {}

            def emit_it1(i, q):
                nc.tensor.matmul(
                    reg(w1, i),
                    lhsT=at[g][:, q, i * P : (i + 1) * P],
                    rhs=zp01[g][:, q, 0:F],
                    start=bank_first[i // 8], stop=(cnt[i] == NT - 1),
                    skip_group_check=True,
                )
                bank_first[i // 8] = False
                cnt[i] += 1

            def dchain(grp):
                lo, hi = 4 * grp, 4 * grp + 4
                tch = dch.tile([P, 4], f32, name="tch", tag="tch")
                uch = dch.tile([P, 4], f32, name="uch", tag="uch")
                wch = dch.tile([P, 4], f32, name="wch", tag="wch")
                rc = rs_ps[:, g * NT + lo : g * NT + hi]
                nc.scalar.activation(tch, rc, Act.Sqrt)
                nc.vector.reciprocal(uch, tch)
                nc.vector.scalar_tensor_tensor(uch, uch, 1.0, rc, Alu.mult, Alu.mult)
                nc.vector.scalar_tensor_tensor(wch, uch, 1.0, tch, Alu.mult, Alu.add)
                nc.vector.reciprocal(uch, wch)   # = d/2
                nc.vector.scalar_tensor_tensor(
                    dsq[:, lo:hi, g], uch, 4.0, uch, Alu.mult, Alu.mult)
                nc.vector.scalar_tensor_tensor(
                    d2s[:, lo:hi, g], uch, 8.0, uch, Alu.mult, Alu.mult)
                nc.vector.tensor_scalar_mul(eh[:, lo:hi, g], wch, 0.25)
                nc.vector.tensor_scalar_mul(dbl[:, lo:hi, g], uch, 4.0)
                uchs[grp] = uch

            def v0_it1(grp):
                lo, hi = 4 * grp, 4 * grp + 4
                uchs.pop(grp)
                # V0 = 2d*X (ACT, per-partition scale)
                for c in range(lo, hi):
                    nc.scalar.activation(
                        zp01[g][:, c, 0:F], xst[g][:, c, :],
                        Act.Copy, scale=dbl[:, c, g : g + 1],
                    )
                for i in range(lo, hi):
                    for q in range(0, lo):
                        emit_it1(i, q)
                for q in range(lo, hi):
                    for i in range(0, hi):
                        emit_it1(i, q)

            for blk in range(NT // CH):
                ach = astage.tile([P, CH, N], bf16, name="ach", tag="ach")
                nc.gpsimd.dma_start(
                    out=ach,
                    in_=a_in[g, blk * CH * P : (blk + 1) * CH * P, :].rearrange(
                        "(k p) j -> p k j", p=P
                    ),
                )
                if blk >= 3 and blk % 2 == 1:
                    v0_it1(blk // 2 - 1)
                for k in range(CH):
                    c = blk * CH + k
                    achc = ach[:, k, :]
                    # zero the diagonal block in place (Pool engine)
                    nc.gpsimd.tensor_mul(
                        achc[:, c * P : (c + 1) * P],
                        achc[:, c * P : (c + 1) * P], mask,
                    )
                    if c in XBAR:
                        nc.sync.dma_start(
                            out=at[g][:, :, c * P : (c + 1) * P],
                            in_=achc, transpose=True,
                        )
                    else:
                        for s4 in range(4):
                            tr = ps_tr.tile([P, 512], f32, name="tr", tag="tr")
                            for j in range(4):
                                q = 4 * s4 + j
                                nc.tensor.matmul(
                                    tr[:, j * P : (j + 1) * P],
                                    lhsT=achc[:, q * P : (q + 1) * P],
                                    rhs=ident,
                                    start=(j == 0), stop=(j == 3),
                                )
                            drain(at[g][:, 4 * s4 : 4 * s4 + 4, c * P : (c + 1) * P], tr)
                    # rowsums ride the PE, two chunks behind the transposes
                    pend_rs.append(c)
                    if len(pend_rs) > 2:
                        emit_rowsum(g, pend_rs.pop(0))
                if blk % 2 == 0 and blk >= 2:
                    dchain(blk // 2 - 1)
            # drain the pipeline tail
            while pend_rs:
                emit_rowsum(g, pend_rs.pop(0))
            dchain(3)
            v0_it1(3)

        # ---- Z^T pair build (projection operand) ----------------------
        def ztbuild(g, zp, ztdst):
            for half in range(4):
                psz = ps_z.tile([P, 512], f32, name="psz", tag="psz")
                for j in range(4):
                    c = 4 * half + j
                    dge = dgep.tile([P, P], bf16, name="dge", tag="dge")
                    nc.scalar.activation(dge, ident, Act.Copy,
                                         scale=eh[:, c, g : g + 1])
                    nc.tensor.matmul(
                        psz[:, j * P : (j + 1) * P],
                        lhsT=zp[g][:, c, :], rhs=dge,
                        start=(j == 0), stop=(j == 3), skip_group_check=True,
                    )
                dst = ztdst[g][:, half * 512 : (half + 1) * 512]
                if half % 2 == 0:
                    nc.scalar.copy(out=dst, in_=psz)
                else:
                    nc.vector.tensor_copy(out=dst, in_=psz)

        # ---- post-load pipeline per graph ------------------------------
        def rest_phase(g):
            w1 = it_ps[g]
            # V1 = d^2 * W0 (ACT, per-partition scale)
            for c in range(NT):
                nc.scalar.activation(
                    zp01[g][:, c, F : 2 * F], reg(w1, c),
                    Act.Copy, scale=dsq[:, c, g : g + 1],
                )
            ztbuild(g, zp01, ztab)
            # iterations 2 and 3
            for t in (2, 3):
                w = [ps_it.tile([P, 512], f32, name=f"w{t}_{g}{h}", tag="it")
                     for h in range(2)]
                rhs_t = zp01[g] if t == 2 else zp23[g]
                rcol = slice(F, 2 * F) if t == 2 else slice(0, F)
                for i in range(NT):
                    for q in range(NT):
                        nc.tensor.matmul(
                            reg(w, i),
                            lhsT=at[g][:, q, i * P : (i + 1) * P],
                            rhs=rhs_t[:, q, rcol],
                            start=(i % 8 == 0 and q == 0),
                            stop=(q == NT - 1),
                            skip_group_check=True,
                        )
                # V_{t+1} = 2d^2 * W_t - V_{t-1}
                dst = slice(0, F) if t == 2 else slice(F, 2 * F)
                prev = slice(0, F) if t == 2 else slice(F, 2 * F)
                for c in range(NT):
                    nc.vector.scalar_tensor_tensor(
                        zp23[g][:, c, dst], reg(w, c),
                        d2s[:, c, g : g + 1], zp01[g][:, c, prev],
                        Alu.mult, Alu.subtract,
                    )
            ztbuild(g, zp23, ztcd)
            # projection + bias + relu + store, one 512-node bank at a time
            for half in range(4):
                pso = ps_z.tile([P, 512], f32, name="pso", tag="psz")
                for j in range(4):
                    c = 4 * half + j
                    r = pso[:, j * OUT : (j + 1) * OUT]
                    nc.tensor.matmul(
                        r, lhsT=ztab[g][:, c * P : (c + 1) * P], rhs=kab,
                        start=(j == 0), stop=False, skip_group_check=True,
                    )
                    nc.tensor.matmul(
                        r, lhsT=ztcd[g][:, c * P : (c + 1) * P], rhs=kcd,
                        start=False, stop=False, skip_group_check=True,
                    )
                nc.tensor.matmul(
                    pso, lhsT=ones_row, rhs=bias4,
                    start=False, stop=True, skip_group_check=True,
                )
                ot = outs.tile([P, 4, OUT], f32, name="ot", tag="ot")
                for j in range(4):
                    src = pso[:, j * OUT : (j + 1) * OUT]
                    if (half + j) % 2 == 0:
                        nc.vector.tensor_scalar_max(ot[:, j, :], src, 0.0)
                    else:
                        nc.scalar.activation(ot[:, j, :], src, Act.Relu)
                nc.sync.dma_start(
                    out=o_out[g, half * 512 : (half + 1) * 512, :].rearrange(
                        "(j p) o -> p j o", p=P
                    ),
                    in_=ot,
                )

        for g in range(GP):
            load_phase(g)
            rest_phase(g)

    nc.finalize()
    return nc


def _get_nc():
    if "nc" not in _cached:
        _cached["nc"] = _build_nc()
    return _cached["nc"]


def kernel(X, A, kernel, bias):
    from concourse.bass_utils import run_bass_kernel_spmd

    nc = _get_nc()
    wk = np.ascontiguousarray(np.asarray(kernel, dtype=np.float32))
    bs = np.ascontiguousarray(np.asarray(bias, dtype=np.float32))
    A = np.asarray(A, dtype=np.float32)
    X = np.asarray(X, dtype=np.float32)
    in_maps = [
        {
            "a": np.ascontiguousarray(A[GP * c : GP * (c + 1)]),
            "x": np.ascontiguousarray(X[GP * c : GP * (c + 1)]),
            "wk": wk,
            "bias": bs,
        }
        for c in range(NCORES)
    ]
    res = run_bass_kernel_spmd(nc, in_maps, core_ids=list(range(NCORES)))
    return np.concatenate([r["out"] for r in res.results], axis=0)


# revision 22
# speedup vs baseline: 1.0881x; 1.0881x over previous
"""ChebConv (K=4) Trainium2 Bass kernel — node-major mapping.

Problem (hardcoded): B=16 graphs, N=2048 nodes, F=64 feats, K=4, out_dim=128.
  L = D A0 D  (A0 = A with zeroed diag, D = diag(1/(eps+sqrt(rowsum(A0)))))
  T0 = X; T1 = L X; T_t = 2 L T_{t-1} - T_{t-2}
  out = relu(concat(T0..T3) @ kernel + bias)

Sharding: batch across 8 cores, 2 graphs per core; host concatenates.

Device algorithm (per core, graphs g=0,1), with V_t := 2d * T_t:
    V0      = 2d*X
    W_t     = A0 @ V_t          (pure bf16 matmul, A^T blocks as weights)
    V1      = d^2 * W0
    V_{t+1} = 2d^2 * W_t - V_{t-1}
    out     = relu( sum_t (V_t/(2d)) @ K_t + bias )

Key layout choice: the Chebyshev matmuls run NODE-major — each output tile
is [128 nodes x 64 feats] with an A^T 128x128 block as the stationary
(lhsT) operand and V as the moving rhs. Outputs are full 128 partitions
wide, so the PE streams half the rows of the feature-major alternative,
and W lands node-major so the V update is a single elementwise op (no
re-transpose). Row sums ride the PE too (ones-column matmuls against the
A^T blocks), freeing the DVE. A arrives f32 in HBM; the SWDGE DMA casts
to bf16 on the fly (2 node-chunks per DMA). A^T is built by identity
matmuls, drained PSUM->SBUF round-robin over ACT/DVE/Pool. The final
projection reads e-scaled Z^T tiles built by diag(e) matmuls; bias is
added with a rank-1 ones x bias matmul and relu rides the PSUM drain.
"""

import numpy as np

P = 128          # partitions
N = 2048         # nodes per graph
F = 64           # input features
OUT = 128        # output features
GP = 2           # graphs per core
NT = N // P      # 16 node chunks
CH = 2           # node chunks per A-load DMA
NCORES = 8

_cached = {}


def _build_nc():
    import ml_dtypes
    import concourse.bacc as bacc
    import concourse.mybir as mybir
    from concourse.tile import TileContext

    f32 = mybir.dt.float32
    bf16 = mybir.dt.bfloat16
    Alu = mybir.AluOpType
    Act = mybir.ActivationFunctionType

    nc = bacc.Bacc("TRN2", target_bir_lowering=False)

    a_in = nc.dram_tensor("a", [GP, N, N], f32, kind="ExternalInput")
    x_in = nc.dram_tensor("x", [GP, N, F], f32, kind="ExternalInput")
    wk_in = nc.dram_tensor("wk", [2 * P, OUT], f32, kind="ExternalInput")
    bias_in = nc.dram_tensor("bias", [OUT], f32, kind="ExternalInput")
    o_out = nc.dram_tensor("out", [GP, N, OUT], f32, kind="ExternalOutput")

    ident_np = np.eye(P, dtype=ml_dtypes.bfloat16)
    ident_dram = nc.inline_tensor(ident_np, name="identbf")

    with TileContext(nc) as tc, \
         tc.tile_pool(name="const", bufs=1) as const, \
         tc.tile_pool(name="big", bufs=1) as big, \
         tc.tile_pool(name="astage", bufs=3) as astage, \
         tc.tile_pool(name="dch", bufs=2) as dch, \
         tc.tile_pool(name="dgep", bufs=16) as dgep, \
         tc.tile_pool(name="outs", bufs=2) as outs, \
         tc.tile_pool(name="ps_tr", bufs=2, space="PSUM") as ps_tr, \
         tc.tile_pool(name="ps_it", bufs=3, space="PSUM") as ps_it, \
         tc.tile_pool(name="ps_rs", bufs=1, space="PSUM") as ps_rs, \
         tc.tile_pool(name="ps_z", bufs=2, space="PSUM") as ps_z:

        # ---- constants -------------------------------------------------
        ident = const.tile([P, P], bf16)
        nc.sync.dma_start(out=ident, in_=ident_dram[:, :])
        mask = const.tile([P, P], bf16)   # 1 - I
        nc.vector.tensor_scalar(mask, ident, -1.0, 1.0, Alu.mult, Alu.add)
        kab = const.tile([P, OUT], bf16)
        kcd = const.tile([P, OUT], bf16)
        kstage = const.tile([P, 2 * OUT], f32)
        nc.sync.dma_start(out=kstage[:, 0:OUT], in_=wk_in[0:P, :])
        nc.sync.dma_start(out=kstage[:, OUT : 2 * OUT], in_=wk_in[P : 2 * P, :])
        nc.vector.tensor_copy(kab, kstage[:, 0:OUT])
        nc.vector.tensor_copy(kcd, kstage[:, OUT : 2 * OUT])
        bias_f32 = const.tile([1, OUT], f32)
        nc.sync.dma_start(out=bias_f32, in_=bias_in[None, :])
        bias4 = const.tile([1, 4 * OUT], bf16)   # bias tiled 4x along free
        for j in range(4):
            nc.vector.tensor_copy(bias4[:, j * OUT : (j + 1) * OUT], bias_f32)
        ones_row = const.tile([1, P], bf16)
        nc.vector.memset(ones_row, 1.0)
        ones_col = const.tile([P, 1], bf16)
        nc.vector.memset(ones_col, 1.0)

        # ---- persistent SBUF state ------------------------------------
        # A^T per graph: [:, q, :] is j-tile q (j = 128q+p), free = node i
        at = [big.tile([P, NT, N], bf16, name=f"at{g}") for g in range(GP)]
        # V pairs, node-major: [:, c, 0:64] = V_t even, [:, c, 64:128] = odd
        zp01 = [big.tile([P, NT, 2 * F], bf16, name=f"zp01_{g}") for g in range(GP)]
        zp23 = [big.tile([P, NT, 2 * F], bf16, name=f"zp23_{g}") for g in range(GP)]
        # e-scaled Z^T pairs for the projection (feature-major)
        ztab = [big.tile([P, N], bf16, name=f"ztab{g}") for g in range(GP)]
        ztcd = [big.tile([P, N], bf16, name=f"ztcd{g}") for g in range(GP)]
        xst = [big.tile([P, NT, F], f32, name=f"xst{g}") for g in range(GP)]
        dsq = big.tile([P, NT, GP], f32, name="dsq")   # d^2
        d2s = big.tile([P, NT, GP], f32, name="d2s")   # 2 d^2
        eh = big.tile([P, NT, GP], f32, name="eh")     # 1/(2d)
        dbl = big.tile([P, NT, GP], f32, name="dbl")   # 2d

        # all 32 per-(graph,chunk) rowsum accumulators live in one bank
        rs_ps = ps_rs.tile([P, 512], f32, name="rsps")

        drain_rr = [0]

        def drain(dst, src):
            k = drain_rr[0] % 2
            drain_rr[0] += 1
            if k == 1:
                nc.vector.tensor_copy(out=dst, in_=src)
            else:
                nc.scalar.copy(out=dst, in_=src)

        def reg(w, i):
            return w[i // 8][:, (i % 8) * F : (i % 8 + 1) * F]

        # PSUM pending-zero is tracked per bank (2KB per partition): only the
        # FIRST write to a bank may carry start=True; later writes to
        # still-pending bytes replace, to cleared bytes accumulate.
        rs_first = [True]

        def emit_rowsum(g, c):
            col = g * NT + c
            for q in range(NT):
                nc.tensor.matmul(
                    rs_ps[:, col : col + 1],
                    lhsT=at[g][:, q, c * P : (c + 1) * P],
                    rhs=ones_col,
                    start=rs_first[0], stop=(q == NT - 1),
                    skip_group_check=True,
                )
                rs_first[0] = False

        it_ps = {}

        # ---- load phase: stream A, cast, transpose, rowsum, d, V0, it1 -
        # Emission is a lagged software pipeline: every consumer is emitted
        # far enough behind its producer that its deps are met at dispatch,
        # so the 4-deep per-engine wait queues never head-of-line block.
        XBAR = set()   # chunks transposed by the DMA XBAR, not PE

        def load_phase(g):
            nc.sync.dma_start(
                out=xst[g], in_=x_in[g].rearrange("(c p) f -> p c f", p=P)
            )
            w1 = [ps_it.tile([P, 512], f32, name=f"w1_{g}{h}", tag="it")
                  for h in range(2)]
            it_ps[g] = w1
            cnt = [0] * NT
            bank_first = [True, True]
            pend_rs = []
            uchs = {}

            def emit_it1(i, q):
                nc.tensor.matmul(
                    reg(w1, i),
                    lhsT=at[g][:, q, i * P : (i + 1) * P],
                    rhs=zp01[g][:, q, 0:F],
                    start=bank_first[i // 8], stop=(cnt[i] == NT - 1),
                    skip_group_check=True,
                )
                bank_first[i // 8] = False
                cnt[i] += 1

            def dchain(grp):
                lo, hi = 4 * grp, 4 * grp + 4
                tch = dch.tile([P, 4], f32, name="tch", tag="tch")
                uch = dch.tile([P, 4], f32, name="uch", tag="uch")
                wch = dch.tile([P, 4], f32, name="wch", tag="wch")
                rc = rs_ps[:, g * NT + lo : g * NT + hi]
                nc.scalar.activation(tch, rc, Act.Sqrt)
                nc.vector.reciprocal(uch, tch)
                nc.vector.scalar_tensor_tensor(uch, uch, 1.0, rc, Alu.mult, Alu.mult)
                nc.vector.scalar_tensor_tensor(wch, uch, 1.0, tch, Alu.mult, Alu.add)
                nc.vector.reciprocal(uch, wch)   # = d/2
                nc.vector.scalar_tensor_tensor(
                    dsq[:, lo:hi, g], uch, 4.0, uch, Alu.mult, Alu.mult)
                nc.vector.scalar_tensor_tensor(
                    d2s[:, lo:hi, g], uch, 8.0, uch, Alu.mult, Alu.mult)
                nc.vector.tensor_scalar_mul(eh[:, lo:hi, g], wch, 0.25)
                nc.vector.tensor_scalar_mul(dbl[:, lo:hi, g], uch, 4.0)
                uchs[grp] = uch

            def v0_it1(grp):
                lo, hi = 4 * grp, 4 * grp + 4
                uchs.pop(grp)
                # V0 = 2d*X (ACT, per-partition scale)
                for c in range(lo, hi):
                    nc.scalar.activation(
                        zp01[g][:, c, 0:F], xst[g][:, c, :],
                        Act.Copy, scale=dbl[:, c, g : g + 1],
                    )
                for i in range(lo, hi):
                    for q in range(0, lo):
                        emit_it1(i, q)
                for q in range(lo, hi):
                    for i in range(0, hi):
                        emit_it1(i, q)

            for blk in range(NT // CH):
                ach = astage.tile([P, CH, N], bf16, name="ach", tag="ach")
                nc.gpsimd.dma_start(
                    out=ach,
                    in_=a_in[g, blk * CH * P : (blk + 1) * CH * P, :].rearrange(
                        "(k p) j -> p k j", p=P
                    ),
                )
                if blk >= 3 and blk % 2 == 1:
                    v0_it1(blk // 2 - 1)
                for k in range(CH):
                    c = blk * CH + k
                    achc = ach[:, k, :]
                    # zero the diagonal block in place (Pool engine)
                    nc.gpsimd.tensor_mul(
                        achc[:, c * P : (c + 1) * P],
                        achc[:, c * P : (c + 1) * P], mask,
                    )
                    if c in XBAR:
                        nc.sync.dma_start(
                            out=at[g][:, :, c * P : (c + 1) * P],
                            in_=achc, transpose=True,
                        )
                    else:
                        for s4 in range(4):
                            tr = ps_tr.tile([P, 512], f32, name="tr", tag="tr")
                            for j in range(4):
                                q = 4 * s4 + j
                                nc.tensor.matmul(
                                    tr[:, j * P : (j + 1) * P],
                                    lhsT=achc[:, q * P : (q + 1) * P],
                                    rhs=ident,
                                    start=(j == 0), stop=(j == 3),
                                )
                            drain(at[g][:, 4 * s4 : 4 * s4 + 4, c * P : (c + 1) * P], tr)
                    # rowsums ride the PE, two chunks behind the transposes
                    pend_rs.append(c)
                    if len(pend_rs) > 2:
                        emit_rowsum(g, pend_rs.pop(0))
                if blk % 2 == 0 and blk >= 2:
                    dchain(blk // 2 - 1)
            # drain the pipeline tail
            while pend_rs:
                emit_rowsum(g, pend_rs.pop(0))
            dchain(3)
            v0_it1(3)

        # ---- Z^T pair build (projection operand) ----------------------
        def build_dges(g):
            dges = []
            for c in range(NT):
                dge = dgep.tile([P, P], bf16, name="dge", tag="dge")
                if c % 2 == 0:
                    nc.scalar.activation(dge, ident, Act.Copy,
                                         scale=eh[:, c, g : g + 1])
                else:
                    nc.vector.tensor_scalar_mul(dge, ident, eh[:, c, g : g + 1])
                dges.append(dge)
            return dges

        def ztbuild(g, zp, ztdst, dges):
            for half in range(4):
                psz = ps_z.tile([P, 512], f32, name="psz", tag="psz")
                for j in range(4):
                    c = 4 * half + j
                    nc.tensor.matmul(
                        psz[:, j * P : (j + 1) * P],
                        lhsT=zp[g][:, c, :], rhs=dges[c],
                        start=(j == 0), stop=(j == 3), skip_group_check=True,
                    )
                dst = ztdst[g][:, half * 512 : (half + 1) * 512]
                if half % 2 == 0:
                    nc.scalar.copy(out=dst, in_=psz)
                else:
                    nc.vector.tensor_copy(out=dst, in_=psz)

        # ---- post-load pipeline per graph ------------------------------
        def rest_phase(g):
            w1 = it_ps[g]
            # V1 = d^2 * W0 (ACT, per-partition scale)
            for c in range(NT):
                nc.scalar.activation(
                    zp01[g][:, c, F : 2 * F], reg(w1, c),
                    Act.Copy, scale=dsq[:, c, g : g + 1],
                )
            dges = build_dges(g)
            ztbuild(g, zp01, ztab, dges)
            # iterations 2 and 3
            for t in (2, 3):
                w = [ps_it.tile([P, 512], f32, name=f"w{t}_{g}{h}", tag="it")
                     for h in range(2)]
                rhs_t = zp01[g] if t == 2 else zp23[g]
                rcol = slice(F, 2 * F) if t == 2 else slice(0, F)
                for i in range(NT):
                    for q in range(NT):
                        nc.tensor.matmul(
                            reg(w, i),
                            lhsT=at[g][:, q, i * P : (i + 1) * P],
                            rhs=rhs_t[:, q, rcol],
                            start=(i % 8 == 0 and q == 0),
                            stop=(q == NT - 1),
                            skip_group_check=True,
                        )
                # V_{t+1} = 2d^2 * W_t - V_{t-1}
                dst = slice(0, F) if t == 2 else slice(F, 2 * F)
                prev = slice(0, F) if t == 2 else slice(F, 2 * F)
                for c in range(NT):
                    nc.vector.scalar_tensor_tensor(
                        zp23[g][:, c, dst], reg(w, c),
                        d2s[:, c, g : g + 1], zp01[g][:, c, prev],
                        Alu.mult, Alu.subtract,
                    )
            ztbuild(g, zp23, ztcd, dges)
            # projection + bias + relu + store, one 512-node bank at a time
            for half in range(4):
                pso = ps_z.tile([P, 512], f32, name="pso", tag="psz")
                for j in range(4):
                    c = 4 * half + j
                    r = pso[:, j * OUT : (j + 1) * OUT]
                    nc.tensor.matmul(
                        r, lhsT=ztab[g][:, c * P : (c + 1) * P], rhs=kab,
                        start=(j == 0), stop=False, skip_group_check=True,
                    )
                    nc.tensor.matmul(
                        r, lhsT=ztcd[g][:, c * P : (c + 1) * P], rhs=kcd,
                        start=False, stop=False, skip_group_check=True,
                    )
                nc.tensor.matmul(
                    pso, lhsT=ones_row, rhs=bias4,
                    start=False, stop=True, skip_group_check=True,
                )
                ot = outs.tile([P, 4, OUT], f32, name="ot", tag="ot")
                for j in range(4):
                    src = pso[:, j * OUT : (j + 1) * OUT]
                    if (half + j) % 2 == 0:
                        nc.vector.tensor_scalar_max(ot[:, j, :], src, 0.0)
                    else:
                        nc.scalar.activation(ot[:, j, :], src, Act.Relu)
                nc.sync.dma_start(
                    out=o_out[g, half * 512 : (half + 1) * 512, :].rearrange(
                        "(j p) o -> p j o", p=P
                    ),
                    in_=ot,
                )

        for g in range(GP):
            load_phase(g)
            rest_phase(g)

    nc.finalize()
    return nc


def _get_nc():
    if "nc" not in _cached:
        _cached["nc"] = _build_nc()
    return _cached["nc"]


def kernel(X, A, kernel, bias):
    from concourse.bass_utils import run_bass_kernel_spmd

    nc = _get_nc()
    wk = np.ascontiguousarray(np.asarray(kernel, dtype=np.float32))
    bs = np.ascontiguousarray(np.asarray(bias, dtype=np.float32))
    A = np.asarray(A, dtype=np.float32)
    X = np.asarray(X, dtype=np.float32)
    in_maps = [
        {
            "a": np.ascontiguousarray(A[GP * c : GP * (c + 1)]),
            "x": np.ascontiguousarray(X[GP * c : GP * (c + 1)]),
            "wk": wk,
            "bias": bs,
        }
        for c in range(NCORES)
    ]
    res = run_bass_kernel_spmd(nc, in_maps, core_ids=list(range(NCORES)))
    return np.concatenate([r["out"] for r in res.results], axis=0)
